# revision 1
# baseline (speedup 1.0000x reference)
"""Trainium2 Bass kernel for the ExemplarHead classification problem (v2, bf16).

Math: per (task, way), with R the 5x1024 class reps (support+noise),
H = I - (1/5)11^T, G = H R R^T H, the SVD head reduces exactly to
    C = W R,  W = I - lam * (lam I + G)^{-1} H
    logits[q,(w,s)] = (2 q.C - ||q||^2 - ||C||^2) / d
(lam I + G) inverse via one scaled Newton step (residual (I-aK)^4 ~ 8e-4,
below the bf16 noise floor). All 20 (task,way) blocks per core are one
masked block-diagonal 100x100 problem.

v2 changes vs v1 (60.4us):
 - all large matmuls in bf16 (1 PE cycle/col vs fp32's 2x2), psum fp32
 - q arrives pre-transposed from host (qT) -> no PE transposes at all
 - one Newton iteration instead of two
 - single packed output DMA; norm folds stay fp32 for accuracy

Sharding: data-parallel over the 32 tasks -> 4 tasks per NeuronCore x 8.
"""

import numpy as np
import ml_dtypes

import concourse.bass as bass
import concourse.mybir as mybir
import concourse.tile as tile
from concourse import bacc
from concourse.bass_utils import run_bass_kernel_spmd

F32 = mybir.dt.float32
BF16 = mybir.dt.bfloat16
AF = mybir.ActivationFunctionType
ALU = mybir.AluOpType

LAM = 100000.0
GMAX_BOUND = 40000.0            # safe bound on ||G|| (observed max ~2.2e4)
ALPHA = 2.0 / (2.0 * LAM + GMAX_BOUND)

N_CORES = 8
T_FULL, NQ, D = 32, 75, 1024
NW, NS = 5, 5
TPC = T_FULL // N_CORES          # tasks per core = 4
NR = TPC * NW * NS               # R rows per core = 100
NCH = D // 128                   # 8 contraction chunks
NJ = NW * NS                     # 25 (way,shot) pairs per task
CF_COLS = 400                    # fp32 const tile columns
CB_COLS = 276                    # bf16 const tile columns


def _host_consts():
    """Packed constant tiles. cF fp32 [128,475], cB bf16 [128,201].

    cF cols: 0:100 alpha*blockmask, 100:200 alpha*lam*I, 200:300 2I,
             300:400 I, col 400:475 ones row (partition 0)
    cB cols: 0:100 H (block-diag), 100:200 alpha*lam*H, col 200 = -0.5
    """
    H5 = np.eye(NS) - np.ones((NS, NS)) / NS
    H_bd = np.kron(np.eye(TPC * NW), H5).astype(np.float32)       # [100,100]
    blockmask = np.kron(np.eye(TPC * NW), np.ones((NS, NS))).astype(np.float32)
    eye = np.eye(NR, dtype=np.float32)
    cF = np.zeros((128, CF_COLS), dtype=np.float32)
    cF[0:NR, 0:NR] = ALPHA * blockmask
    cF[0:NR, NR:2 * NR] = ALPHA * LAM * eye
    cF[0:NR, 2 * NR:3 * NR] = 2.0 * eye
    cF[0:NR, 3 * NR:4 * NR] = eye
    cB = np.zeros((128, CB_COLS), dtype=np.float32)
    cB[0:NR, 0:NR] = H_bd
    cB[0:NR, NR:2 * NR] = ALPHA * LAM * H_bd
    cB[:, 200] = -0.5
    cB[0, 201:201 + NQ] = 1.0
    return cF, cB.astype(ml_dtypes.bfloat16)


def build_nc():
    nc = bacc.Bacc("TRN2")

    qt_d = nc.declare_dram_parameter("qt", [NCH, 128, TPC * NQ], BF16,
                                     isOutput=False)
    qn_d = nc.declare_dram_parameter("qn", [NQ, TPC * D], BF16, isOutput=False)
    sn_d = nc.declare_dram_parameter("sn", [NR, D], F32, isOutput=False)
    nz_d = nc.declare_dram_parameter("nz", [NR, D], F32, isOutput=False)
    cF_d = nc.declare_dram_parameter("cF", [128, CF_COLS], F32, isOutput=False)
    cB_d = nc.declare_dram_parameter("cB", [128, CB_COLS], BF16, isOutput=False)
    out_d = nc.declare_dram_parameter("out", [NQ, TPC * NJ], F32, isOutput=True)

    with tile.TileContext(nc) as tc:
        with (
            tc.tile_pool(name="consts", bufs=1) as consts,
            tc.tile_pool(name="sb", bufs=1) as sb,
            tc.tile_pool(name="scr", bufs=2) as scr,
            tc.tile_pool(name="pipe", bufs=3, space="PSUM") as pipe,
            tc.tile_pool(name="gp", bufs=1, space="PSUM") as gp,
            tc.tile_pool(name="cnp", bufs=1, space="PSUM") as cnp,
            tc.tile_pool(name="qcp", bufs=2, space="PSUM") as qcp,
        ):
            # ---- R inputs first on the SP HWDGE ring (gate the PE) ----
            sn_sb = sb.tile([NR, D], F32)
            nz_sb = sb.tile([NR, D], F32)
            cB = consts.tile([128, CB_COLS], BF16)
            cF = consts.tile([128, CF_COLS], F32)
            HD = D // 2
            nc.sync.dma_start(out=sn_sb[:, 0:HD], in_=sn_d[:, 0:HD])
            nc.sync.dma_start(out=nz_sb[:, 0:HD], in_=nz_d[:, 0:HD])
            nc.sync.dma_start(out=cB, in_=cB_d[:])
            nc.sync.dma_start(out=sn_sb[:, HD:D], in_=sn_d[:, HD:D])
            nc.sync.dma_start(out=nz_sb[:, HD:D], in_=nz_d[:, HD:D])
            nc.sync.dma_start(out=cF, in_=cF_d[:])
            c_amask = cF[0:NR, 0:NR]
            c_alI = cF[0:NR, NR:2 * NR]
            c_2I = cF[0:NR, 2 * NR:3 * NR]
            c_I = cF[0:NR, 3 * NR:4 * NR]
            c_Hb = cB[0:NR, 0:NR]
            c_alHb = cB[0:NR, NR:2 * NR]
            neghb = cB[:, 200:201]
            ones75b = cB[0:1, 201:201 + NQ]

            # ---- q loads on the second (Activation) HWDGE ring ----
            qtb = sb.tile([128, NCH * TPC * NQ], BF16)
            for k in range(NCH):
                nc.scalar.dma_start(out=qtb[:, k * 300:(k + 1) * 300],
                                    in_=qt_d[k])
            qn_nat = sb.tile([NQ, TPC * D], BF16)
            nc.scalar.dma_start(out=qn_nat, in_=qn_d[:])

            # ---- R = support + noise on DVE (fuses the bf16 cast) ----
            rb = sb.tile([NR, D], BF16)
            for h in range(2):
                sl = slice(h * HD, (h + 1) * HD)
                nc.vector.tensor_add(rb[:, sl], sn_sb[:, sl], nz_sb[:, sl])

            # early DVE touch so later DVE ops don't re-wait the const sems
            warm = sb.tile([1, 2], F32)
            nc.vector.tensor_copy(warm[0:1, 0:1], cF[0:1, 0:1])
            nc.vector.tensor_copy(warm[0:1, 1:2], cB[0:1, 0:1])

            # ---- ||q||^2 per task (scalar engine, overlaps PE phase) ----
            qnorm = sb.tile([NQ, TPC], F32)
            qbias = sb.tile([NQ, TPC], F32)
            for t in range(TPC):
                sq_scr = scr.tile([NQ, D], BF16, tag="sq")
                nc.scalar.activation(sq_scr, qn_nat[:, t * D:(t + 1) * D],
                                     AF.Square, accum_out=qnorm[:, t:t + 1])
            nc.scalar.activation(qbias, qnorm, AF.Copy, scale=-1.0 / D)

            # ---- RcT = (H R)^T by chunks (bf16) ----
            rctb = sb.tile([128, NCH * NR], BF16)
            for p in range(2):
                rct_ps = pipe.tile([128, 4 * NR], F32, space="PSUM", tag="pp")
                for kk in range(4):
                    k = 4 * p + kk
                    nc.tensor.matmul(rct_ps[:, kk * NR:(kk + 1) * NR],
                                     lhsT=rb[:, k * 128:(k + 1) * 128],
                                     rhs=c_Hb, start=True, stop=True)
                nc.vector.tensor_copy(rctb[:, p * 4 * NR:(p + 1) * 4 * NR],
                                      rct_ps)

            # ---- G = sum_k RcT_k^T RcT_k ----
            g_ps = gp.tile([NR, NR], F32, space="PSUM")
            for k in range(NCH):
                rct_k = rctb[:, k * NR:(k + 1) * NR]
                nc.tensor.matmul(g_ps, lhsT=rct_k, rhs=rct_k,
                                 start=(k == 0), stop=(k == NCH - 1))

            # ---- K_alpha, one Newton step, W^T ----
            gm_f = sb.tile([NR, NR], F32)
            nc.vector.tensor_mul(gm_f, g_ps, c_amask)
            ka_f = sb.tile([NR, NR], F32)
            nc.vector.tensor_add(ka_f, gm_f, c_alI)
            ka_b = sb.tile([NR, NR], BF16)
            nc.scalar.copy(ka_b, ka_f)                     # ACT, overlaps DVE
            y1_b = sb.tile([NR, NR], BF16)
            nc.vector.tensor_sub(y1_b, c_2I, ka_f)         # Y1 = 2I - Ka
            p_ps = pipe.tile([NR, NR], F32, space="PSUM", tag="pp")
            nc.tensor.matmul(p_ps, lhsT=ka_b, rhs=y1_b, start=True, stop=True)
            qq_b = sb.tile([NR, NR], BF16)
            nc.vector.tensor_sub(qq_b, c_2I, p_ps)         # 2I - Ka Y1
            y2_ps = pipe.tile([NR, NR], F32, space="PSUM", tag="pp")
            nc.tensor.matmul(y2_ps, lhsT=y1_b, rhs=qq_b, start=True, stop=True)
            y2_b = sb.tile([NR, NR], BF16)
            nc.scalar.copy(y2_b, y2_ps)
            hy_ps = pipe.tile([NR, NR], F32, space="PSUM", tag="pp")
            nc.tensor.matmul(hy_ps, lhsT=c_alHb, rhs=y2_b, start=True,
                             stop=True)
            wt_b = sb.tile([NR, NR], BF16)
            nc.vector.tensor_sub(wt_b, c_I, hy_ps)         # W^T = I - alH Y

            # ---- C^T chunks (bf16) + squares for ||C||^2 ----
            ctb = sb.tile([128, NCH * NR], BF16)
            csqb = sb.tile([128, NCH * NR], BF16)
            for p in range(2):
                ct_ps = pipe.tile([128, 4 * NR], F32, space="PSUM", tag="pp")
                for kk in range(4):
                    k = 4 * p + kk
                    nc.tensor.matmul(ct_ps[:, kk * NR:(kk + 1) * NR],
                                     lhsT=rb[:, k * 128:(k + 1) * 128],
                                     rhs=wt_b, start=True, stop=True)
                sl = slice(p * 4 * NR, (p + 1) * 4 * NR)
                nc.vector.tensor_copy(ctb[:, sl], ct_ps)
                nc.scalar.activation(csqb[:, sl], ct_ps, AF.Square)

            # ---- cn row: [1,100] = sum_d -0.5 * C^T(d,j)^2 (fp32 result) ----
            cn_ps = cnp.tile([1, NR], F32, space="PSUM")
            for k in range(NCH):
                nc.tensor.matmul(cn_ps, lhsT=neghb,
                                 rhs=csqb[:, k * NR:(k + 1) * NR],
                                 start=(k == 0), stop=(k == NCH - 1))
            cn_f = sb.tile([1, NR], F32)
            nc.scalar.copy(cn_f, cn_ps)
            cnh_b = sb.tile([1, NR], BF16)
            nc.scalar.copy(cnh_b, cn_ps)
            cnh_f = sb.tile([1, NR], F32)
            nc.scalar.copy(cnh_f, cnh_b)
            cnr_b = sb.tile([1, NR], BF16)
            nc.vector.tensor_sub(cnr_b, cn_f, cnh_f)

            # ---- QC per task + fp32 rank-1 cn fold + fused epilogue ----
            out_sb = sb.tile([NQ, TPC * NJ], F32)
            for t in range(TPC):
                qc_ps = qcp.tile([NQ, NJ], F32, space="PSUM", tag="qc",
                                 name=f"qc{t}")
                for k in range(NCH):
                    lhs = qtb[:, k * 300 + t * NQ:k * 300 + (t + 1) * NQ]
                    rhs = ctb[:, k * NR + t * NJ:k * NR + t * NJ + NJ]
                    nc.tensor.matmul(qc_ps, lhsT=lhs, rhs=rhs,
                                     start=(k == 0), stop=False)
                nc.tensor.matmul(qc_ps, lhsT=ones75b,
                                 rhs=cnh_b[0:1, t * NJ:(t + 1) * NJ],
                                 start=False, stop=False)
                nc.tensor.matmul(qc_ps, lhsT=ones75b,
                                 rhs=cnr_b[0:1, t * NJ:(t + 1) * NJ],
                                 start=False, stop=True)
                # logits = (2/D)*psum + (-qn/D), one dual-op DVE instr
                nc.vector.tensor_scalar(out_sb[:, t * NJ:(t + 1) * NJ],
                                        qc_ps, 2.0 / D, qbias[:, t:t + 1],
                                        ALU.mult, ALU.add)
            nc.sync.dma_start(out=out_d[:], in_=out_sb)

    nc.finalize()
    return nc


_NC_CACHE = None


def _get_nc():
    global _NC_CACHE
    if _NC_CACHE is None:
        _NC_CACHE = build_nc()
    return _NC_CACHE


def make_in_maps(query, support, noise):
    query = np.asarray(query, dtype=np.float32)
    support = np.asarray(support, dtype=np.float32)
    noise = np.asarray(noise, dtype=np.float32)
    cF, cB = _host_consts()
    in_maps = []
    for c in range(N_CORES):
        ts = slice(c * TPC, (c + 1) * TPC)
        qc = query[ts]                                   # (4, 75, 1024)
        qt = np.ascontiguousarray(
            qc.transpose(2, 0, 1).reshape(NCH, 128, TPC * NQ)
        ).astype(ml_dtypes.bfloat16)
        qn = np.ascontiguousarray(
            qc.transpose(1, 0, 2).reshape(NQ, TPC * D)
        ).astype(ml_dtypes.bfloat16)
        in_maps.append({
            "qt": qt,
            "qn": qn,
            "sn": np.ascontiguousarray(support[ts]).reshape(NR, D),
            "nz": np.ascontiguousarray(
                noise[:, ts].transpose(1, 0, 2, 3)).reshape(NR, D),
            "cF": cF,
            "cB": cB,
        })
    return in_maps


def kernel(query, support, noise, support_labels=None, n_way=None, n_shot=None,
           **_unused):
    nc = _get_nc()
    in_maps = make_in_maps(query, support, noise)
    res = run_bass_kernel_spmd(nc, in_maps, list(range(N_CORES)))
    outs = [np.asarray(r["out"]).reshape(NQ, TPC, NJ).transpose(1, 0, 2)
            for r in res.results]
    full = np.concatenate(outs, axis=0)            # (32, 75, 25)
    return full.reshape(T_FULL, NQ, NW, NS).astype(np.float32)



# revision 5
# speedup vs baseline: 1.3367x; 1.3367x over previous
"""Trainium2 Bass kernel for the ExemplarHead classification problem (v3).

Math: per (task, way), with R the 5x1024 class reps (support+noise),
H = I - (1/5)11^T, G = H R R^T H, the SVD head reduces exactly to
    C = W R,  W = I - lam * (lam I + G)^{-1} H
    logits[q,(w,s)] = (2 q.C - ||q||^2 - ||C||^2) / d
(lam I + G) inverse via one scaled Newton step. All 20 (task,way) blocks
per core are one masked block-diagonal 100x100 problem.

v3 changes vs v2 (44.1us measured):
 - QC phase transposed: psum[(t,j),(t,q)] = C q^T as 8 matmuls of N=300
   into one [100,300] PSUM bank (was 40 matmuls of N=25); norms folded
   as rank-1 matmuls (rows for ||C||^2, cols for ||q||^2).
 - qn input eliminated; ||q||^2 = ones^T (qt.^2) on device (1 big Square
   + 8 M=1 matmuls) -> Scalar engine no longer blocks the Newton chain.
 - sn/nz uploaded bf16 (halves R DMA); qt coalesced to ONE DMA.
 - PE warmup burst at t=0 so the HAM clock gate is released (2.4 GHz)
   before the real matmul stream begins.

Sharding: data-parallel over the 32 tasks -> 4 tasks per NeuronCore x 8.
"""

import numpy as np
import ml_dtypes

import concourse.bass as bass
import concourse.mybir as mybir
import concourse.tile as tile
from concourse import bacc
from concourse.bass_utils import run_bass_kernel_spmd

F32 = mybir.dt.float32
BF16 = mybir.dt.bfloat16
AF = mybir.ActivationFunctionType
ALU = mybir.AluOpType

LAM = 100000.0
GMAX_BOUND = 40000.0            # safe bound on ||G|| (observed max ~2.2e4)
ALPHA = 2.0 / (2.0 * LAM + GMAX_BOUND)

N_CORES = 8
T_FULL, NQ, D = 32, 75, 1024
NW, NS = 5, 5
TPC = T_FULL // N_CORES          # tasks per core = 4
NR = TPC * NW * NS               # R rows per core = 100
NCH = D // 128                   # 8 contraction chunks
NJ = NW * NS                     # 25 (way,shot) pairs per task
NQT = TPC * NQ                   # 300 (task,query) columns per core
CF_COLS = 400                    # fp32 const tile columns
CB_COLS = 502                    # bf16 const tile columns
N_WARM = 30                      # PE warmup matmuls
WN = 384                         # warmup matmul free size


def _host_consts():
    """Packed constant tiles. cF fp32 [128,400], cB bf16 [128,502].

    cF cols: 0:100 alpha*blockmask, 100:200 alpha*lam*I, 200:300 2I,
             300:400 I
    cB cols: 0:100 H (block-diag), 100:200 alpha*lam*H,
             col 200 = -0.5 (full column), col 201 = 1.0 (full column),
             cols 202:502 = 1.0 on partition 0 (ones row)
    """
    H5 = np.eye(NS) - np.ones((NS, NS)) / NS
    H_bd = np.kron(np.eye(TPC * NW), H5).astype(np.float32)       # [100,100]
    blockmask = np.kron(np.eye(TPC * NW), np.ones((NS, NS))).astype(np.float32)
    eye = np.eye(NR, dtype=np.float32)
    cF = np.zeros((128, CF_COLS), dtype=np.float32)
    cF[0:NR, 0:NR] = ALPHA * blockmask
    cF[0:NR, NR:2 * NR] = ALPHA * LAM * eye
    cF[0:NR, 2 * NR:3 * NR] = 2.0 * eye
    cF[0:NR, 3 * NR:4 * NR] = eye
    cB = np.zeros((128, CB_COLS), dtype=np.float32)
    cB[0:NR, 0:NR] = H_bd
    cB[0:NR, NR:2 * NR] = ALPHA * LAM * H_bd
    cB[:, 200] = -0.5
    cB[:, 201] = 1.0
    cB[0, 202:202 + NQT] = 1.0
    return cF, cB.astype(ml_dtypes.bfloat16)


def build_nc():
    nc = bacc.Bacc("TRN2")

    qt_d = nc.declare_dram_parameter("qt", [128, NCH * NQT], BF16,
                                     isOutput=False)
    sn_d = nc.declare_dram_parameter("sn", [NR, D], BF16, isOutput=False)
    nz_d = nc.declare_dram_parameter("nz", [NR, D], BF16, isOutput=False)
    cF_d = nc.declare_dram_parameter("cF", [128, CF_COLS], F32, isOutput=False)
    cB_d = nc.declare_dram_parameter("cB", [128, CB_COLS], BF16, isOutput=False)
    out_d = nc.declare_dram_parameter("out", [NR, NQT], F32, isOutput=True)

    with tile.TileContext(nc) as tc:
        with (
            tc.tile_pool(name="consts", bufs=1) as consts,
            tc.tile_pool(name="sb", bufs=1) as sb,
            tc.tile_pool(name="scr", bufs=2) as scr,
            tc.tile_pool(name="pipe", bufs=3, space="PSUM") as pipe,
            tc.tile_pool(name="gp", bufs=1, space="PSUM") as gp,
            tc.tile_pool(name="cnp", bufs=1, space="PSUM") as cnp,
            tc.tile_pool(name="qnp", bufs=1, space="PSUM") as qnp,
            tc.tile_pool(name="qcp", bufs=1, space="PSUM") as qcp,
            tc.tile_pool(name="wp", bufs=1, space="PSUM") as wp,
        ):
            # ---- R-path inputs on the SP HWDGE ring (gate the PE) ----
            cB = consts.tile([128, CB_COLS], BF16)
            sn_sb = sb.tile([NR, D], BF16)
            nz_sb = sb.tile([NR, D], BF16)
            cF = consts.tile([128, CF_COLS], F32)
            nc.sync.dma_start(out=cB, in_=cB_d[:])
            nc.sync.dma_start(out=sn_sb, in_=sn_d[:])
            nc.sync.dma_start(out=nz_sb, in_=nz_d[:])
            nc.sync.dma_start(out=cF, in_=cF_d[:])
            c_amask = cF[0:NR, 0:NR]
            c_alI = cF[0:NR, NR:2 * NR]
            c_2I = cF[0:NR, 2 * NR:3 * NR]
            c_I = cF[0:NR, 3 * NR:4 * NR]
            c_Hb = cB[0:NR, 0:NR]
            c_alHb = cB[0:NR, NR:2 * NR]
            neghb = cB[:, 200:201]
            onescol = cB[:, 201:202]
            ones100 = cB[0:1, 202:202 + NR]
            ones300 = cB[0:1, 202:202 + NQT]

            # ---- q load: one coalesced DMA on the Activation HWDGE ring ----
            qtb = sb.tile([128, NCH * NQT], BF16)
            nc.scalar.dma_start(out=qtb, in_=qt_d[:])

            # ---- PE warmup: release the HAM clock gate before real work ----
            wsrc = sb.tile([128, WN], BF16)
            nc.vector.memset(wsrc, 0.0)
            w_ps = wp.tile([128, WN], F32, space="PSUM")
            for i in range(N_WARM):
                nc.tensor.matmul(w_ps, lhsT=wsrc[:, 0:128], rhs=wsrc,
                                 start=True, stop=True)

            # ---- R = support + noise on DVE (bf16) ----
            HD = D // 2
            rb = sb.tile([NR, D], BF16)
            for h in range(2):
                sl = slice(h * HD, (h + 1) * HD)
                nc.vector.tensor_add(rb[:, sl], sn_sb[:, sl], nz_sb[:, sl])

            # early DVE touch so later DVE ops don't re-wait the const sems
            warm = sb.tile([1, 2], F32)
            nc.vector.tensor_copy(warm[0:1, 0:1], cF[0:1, 0:1])
            nc.vector.tensor_copy(warm[0:1, 1:2], cB[0:1, 0:1])

            # ---- sq = qt.^2 (Scalar, 2 halves) for ||q||^2 ----
            sq = sb.tile([128, NCH * NQT], BF16)
            HQ = NCH * NQT // 2
            for h in range(2):
                sl = slice(h * HQ, (h + 1) * HQ)
                nc.scalar.activation(sq[:, sl], qtb[:, sl], AF.Square)

            # ---- RcT = (H R)^T by chunks (bf16) ----
            rctb = sb.tile([128, NCH * NR], BF16)
            for p in range(2):
                rct_ps = pipe.tile([128, 4 * NR], F32, space="PSUM", tag="pp")
                for kk in range(4):
                    k = 4 * p + kk
                    nc.tensor.matmul(rct_ps[:, kk * NR:(kk + 1) * NR],
                                     lhsT=rb[:, k * 128:(k + 1) * 128],
                                     rhs=c_Hb, start=True, stop=True)
                nc.vector.tensor_copy(rctb[:, p * 4 * NR:(p + 1) * 4 * NR],
                                      rct_ps)

            # ---- G = sum_k RcT_k^T RcT_k ----
            g_ps = gp.tile([NR, NR], F32, space="PSUM")
            for k in range(NCH):
                rct_k = rctb[:, k * NR:(k + 1) * NR]
                nc.tensor.matmul(g_ps, lhsT=rct_k, rhs=rct_k,
                                 start=(k == 0), stop=(k == NCH - 1))

            # ---- qn2 = -0.5*||q||^2 row [1,300] via ones^T sq ----
            qn_ps = qnp.tile([1, NQT], F32, space="PSUM")
            for k in range(NCH):
                nc.tensor.matmul(qn_ps, lhsT=onescol,
                                 rhs=sq[:, k * NQT:(k + 1) * NQT],
                                 start=(k == 0), stop=(k == NCH - 1))
            qn_f = sb.tile([1, NQT], F32)
            nc.scalar.activation(qn_f, qn_ps, AF.Copy, scale=-0.5)
            qnh_b = sb.tile([1, NQT], BF16)
            nc.scalar.activation(qnh_b, qn_ps, AF.Copy, scale=-0.5)
            qnh_f = sb.tile([1, NQT], F32)
            nc.scalar.copy(qnh_f, qnh_b)
            qnr_b = sb.tile([1, NQT], BF16)
            nc.vector.tensor_sub(qnr_b, qn_f, qnh_f)

            # ---- K_alpha, one Newton step, W^T ----
            gm_f = sb.tile([NR, NR], F32)
            nc.vector.tensor_mul(gm_f, g_ps, c_amask)
            ka_f = sb.tile([NR, NR], F32)
            nc.vector.tensor_add(ka_f, gm_f, c_alI)
            ka_b = sb.tile([NR, NR], BF16)
            nc.scalar.copy(ka_b, ka_f)                     # ACT, overlaps DVE
            y1_b = sb.tile([NR, NR], BF16)
            nc.vector.tensor_sub(y1_b, c_2I, ka_f)         # Y1 = 2I - Ka
            p_ps = pipe.tile([NR, NR], F32, space="PSUM", tag="pp")
            nc.tensor.matmul(p_ps, lhsT=ka_b, rhs=y1_b, start=True, stop=True)
            qq_b = sb.tile([NR, NR], BF16)
            nc.vector.tensor_sub(qq_b, c_2I, p_ps)         # 2I - Ka Y1
            y2_ps = pipe.tile([NR, NR], F32, space="PSUM", tag="pp")
            nc.tensor.matmul(y2_ps, lhsT=y1_b, rhs=qq_b, start=True, stop=True)
            y2_b = sb.tile([NR, NR], BF16)
            nc.scalar.copy(y2_b, y2_ps)
            hy_ps = pipe.tile([NR, NR], F32, space="PSUM", tag="pp")
            nc.tensor.matmul(hy_ps, lhsT=c_alHb, rhs=y2_b, start=True,
                             stop=True)
            wt_b = sb.tile([NR, NR], BF16)
            nc.vector.tensor_sub(wt_b, c_I, hy_ps)         # W^T = I - alH Y

            # ---- C^T chunks (bf16) + squares for ||C||^2 ----
            ctb = sb.tile([128, NCH * NR], BF16)
            csqb = sb.tile([128, NCH * NR], BF16)
            for p in range(2):
                ct_ps = pipe.tile([128, 4 * NR], F32, space="PSUM", tag="pp")
                for kk in range(4):
                    k = 4 * p + kk
                    nc.tensor.matmul(ct_ps[:, kk * NR:(kk + 1) * NR],
                                     lhsT=rb[:, k * 128:(k + 1) * 128],
                                     rhs=wt_b, start=True, stop=True)
                sl = slice(p * 4 * NR, (p + 1) * 4 * NR)
                nc.vector.tensor_copy(ctb[:, sl], ct_ps)
                nc.scalar.activation(csqb[:, sl], ct_ps, AF.Square)

            # ---- cn row: [1,100] = sum_d -0.5 * C^T(d,j)^2 (fp32) ----
            cn_ps = cnp.tile([1, NR], F32, space="PSUM")
            for k in range(NCH):
                nc.tensor.matmul(cn_ps, lhsT=neghb,
                                 rhs=csqb[:, k * NR:(k + 1) * NR],
                                 start=(k == 0), stop=(k == NCH - 1))
            cn_f = sb.tile([1, NR], F32)
            nc.scalar.copy(cn_f, cn_ps)
            cnh_b = sb.tile([1, NR], BF16)
            nc.scalar.copy(cnh_b, cn_ps)
            cnh_f = sb.tile([1, NR], F32)
            nc.scalar.copy(cnh_f, cnh_b)
            cnr_b = sb.tile([1, NR], BF16)
            nc.vector.tensor_sub(cnr_b, cn_f, cnh_f)

            # ---- QC transposed: psum[(t,j),(t,q)] = C q^T + norm folds ----
            qc_ps = qcp.tile([NR, NQT], F32, space="PSUM")
            nc.tensor.matmul(qc_ps, lhsT=ones100, rhs=qnh_b,
                             start=True, stop=False)
            nc.tensor.matmul(qc_ps, lhsT=ones100, rhs=qnr_b,
                             start=False, stop=False)
            for k in range(NCH):
                nc.tensor.matmul(qc_ps, lhsT=ctb[:, k * NR:(k + 1) * NR],
                                 rhs=qtb[:, k * NQT:(k + 1) * NQT],
                                 start=False, stop=False)
            nc.tensor.matmul(qc_ps, lhsT=cnh_b, rhs=ones300,
                             start=False, stop=False)
            nc.tensor.matmul(qc_ps, lhsT=cnr_b, rhs=ones300,
                             start=False, stop=True)

            # ---- epilogue: scale full psum, DMA out; host slices blocks ----
            out_sb = sb.tile([NR, NQT], F32)
            nc.vector.tensor_scalar(out_sb, qc_ps, 2.0 / D, None, ALU.mult)
            nc.sync.dma_start(out=out_d[:], in_=out_sb)

    nc.finalize()
    return nc


_NC_CACHE = None


def _get_nc():
    global _NC_CACHE
    if _NC_CACHE is None:
        _NC_CACHE = build_nc()
    return _NC_CACHE


def make_in_maps(query, support, noise):
    query = np.asarray(query, dtype=np.float32)
    support = np.asarray(support, dtype=np.float32)
    noise = np.asarray(noise, dtype=np.float32)
    cF, cB = _host_consts()
    in_maps = []
    for c in range(N_CORES):
        ts = slice(c * TPC, (c + 1) * TPC)
        qc = query[ts]                                   # (4, 75, 1024)
        # qt[p, k*300 + t*75 + q] = q[t, q, 128k+p]
        qt = np.ascontiguousarray(
            qc.transpose(2, 0, 1).reshape(NCH, 128, NQT)
              .transpose(1, 0, 2).reshape(128, NCH * NQT)
        ).astype(ml_dtypes.bfloat16)
        in_maps.append({
            "qt": qt,
            "sn": np.ascontiguousarray(support[ts]).reshape(NR, D)
                  .astype(ml_dtypes.bfloat16),
            "nz": np.ascontiguousarray(
                noise[:, ts].transpose(1, 0, 2, 3)).reshape(NR, D)
                  .astype(ml_dtypes.bfloat16),
            "cF": cF,
            "cB": cB,
        })
    return in_maps


def kernel(query, support, noise, support_labels=None, n_way=None, n_shot=None,
           **_unused):
    nc = _get_nc()
    in_maps = make_in_maps(query, support, noise)
    res = run_bass_kernel_spmd(nc, in_maps, list(range(N_CORES)))
    # out is [(t,j), (t',q)] = [100, 300]; take diagonal task blocks,
    # then (4, 25, 75) -> (4, 75, 25)
    outs = []
    for r in res.results:
        o = np.asarray(r["out"]).reshape(TPC, NJ, TPC, NQ)
        blk = o[np.arange(TPC), :, np.arange(TPC), :]   # (4, 25, 75)
        outs.append(blk.transpose(0, 2, 1))
    full = np.concatenate(outs, axis=0)            # (32, 75, 25)
    return full.reshape(T_FULL, NQ, NW, NS).astype(np.float32)


# revision 6
# speedup vs baseline: 1.4051x; 1.0512x over previous
"""Trainium2 Bass kernel for the ExemplarHead classification problem (v4).

Math: per (task, way), with R the 5x1024 class reps (support+noise),
H = I - (1/5)11^T, G = H R R^T H, the SVD head reduces exactly to
    C = W R,  W = I - lam * (lam I + G)^{-1} H
    logits[q,(w,s)] = (2 q.C - ||q||^2 - ||C||^2) / d
(lam I + G) inverse via one scaled Newton step. All 20 (task,way) blocks
per core are one masked block-diagonal 100x100 problem.

v4 changes vs v3 (33.0us measured):
 - qt shipped as fp8e4m3 and cast to bf16 by a SWDGE (gpsimd-ring) DMA:
   half the bytes, third parallel DMA path.
 - consts shrunk to one [100,400] bf16 tile (H, al*lam*H, I, amask);
   -0.5/1.0 columns+rows from memset, diag(316)/diag(12)/2I derived
   on-device from I.
 - lam*I folded into the G psum via two exact diagonal matmuls
   (316^2 + 12^2 = 100000 exactly) -> two fewer DVE hops pre-Newton.
 - cn residual split runs DVE || ACT instead of a serial chain.

Sharding: data-parallel over the 32 tasks -> 4 tasks per NeuronCore x 8.
"""

import numpy as np
import ml_dtypes

import concourse.bass as bass
import concourse.mybir as mybir
import concourse.tile as tile
from concourse import bacc
from concourse.bass_utils import run_bass_kernel_spmd

F32 = mybir.dt.float32
BF16 = mybir.dt.bfloat16
FP8 = mybir.dt.float8e4
AF = mybir.ActivationFunctionType
ALU = mybir.AluOpType

LAM = 100000.0
GMAX_BOUND = 40000.0            # safe bound on ||G|| (observed max ~2.2e4)
ALPHA = 2.0 / (2.0 * LAM + GMAX_BOUND)

N_CORES = 8
T_FULL, NQ, D = 32, 75, 1024
NW, NS = 5, 5
TPC = T_FULL // N_CORES          # tasks per core = 4
NR = TPC * NW * NS               # R rows per core = 100
NCH = D // 128                   # 8 contraction chunks
NJ = NW * NS                     # 25 (way,shot) pairs per task
NQT = TPC * NQ                   # 300 (task,query) columns per core
CB_COLS = 400                    # bf16 const tile columns
N_WARM = 11                      # PE warmup matmuls
WN = 384                         # warmup matmul free size


def _host_consts():
    """cB bf16 [100,400]: H (block-diag), alpha*lam*H, I, alpha*blockmask."""
    H5 = np.eye(NS) - np.ones((NS, NS)) / NS
    H_bd = np.kron(np.eye(TPC * NW), H5).astype(np.float32)       # [100,100]
    blockmask = np.kron(np.eye(TPC * NW), np.ones((NS, NS))).astype(np.float32)
    eye = np.eye(NR, dtype=np.float32)
    cB = np.zeros((NR, CB_COLS), dtype=np.float32)
    cB[:, 0:NR] = H_bd
    cB[:, NR:2 * NR] = ALPHA * LAM * H_bd
    cB[:, 2 * NR:3 * NR] = eye
    cB[:, 3 * NR:4 * NR] = ALPHA * blockmask
    return cB.astype(ml_dtypes.bfloat16)


def build_nc():
    nc = bacc.Bacc("TRN2")

    qt_d = nc.declare_dram_parameter("qt", [128, NCH * NQT], FP8,
                                     isOutput=False)
    sn_d = nc.declare_dram_parameter("sn", [NR, D], BF16, isOutput=False)
    nz_d = nc.declare_dram_parameter("nz", [NR, D], BF16, isOutput=False)
    cB_d = nc.declare_dram_parameter("cB", [NR, CB_COLS], BF16, isOutput=False)
    out_d = nc.declare_dram_parameter("out", [NR, NQT], F32, isOutput=True)

    with tile.TileContext(nc) as tc:
        with (
            tc.tile_pool(name="consts", bufs=1) as consts,
            tc.tile_pool(name="sb", bufs=1) as sb,
            tc.tile_pool(name="pipe", bufs=3, space="PSUM") as pipe,
            tc.tile_pool(name="gp", bufs=1, space="PSUM") as gp,
            tc.tile_pool(name="cnp", bufs=1, space="PSUM") as cnp,
            tc.tile_pool(name="qnp", bufs=1, space="PSUM") as qnp,
            tc.tile_pool(name="qcp", bufs=1, space="PSUM") as qcp,
            tc.tile_pool(name="wp", bufs=1, space="PSUM") as wp,
        ):
            # ---- input DMAs: 3 parallel paths ----
            cB = consts.tile([NR, CB_COLS], BF16)
            sn_sb = sb.tile([NR, D], BF16)
            nz_sb = sb.tile([NR, D], BF16)
            nc.sync.dma_start(out=cB, in_=cB_d[:])
            nc.sync.dma_start(out=sn_sb, in_=sn_d[:])
            nc.sync.dma_start(out=nz_sb, in_=nz_d[:])
            qtb = sb.tile([128, NCH * NQT], BF16)
            nc.gpsimd.dma_start(out=qtb, in_=qt_d[:])      # SWDGE fp8->bf16
            c_Hb = cB[:, 0:NR]
            c_alHb = cB[:, NR:2 * NR]
            c_I = cB[:, 2 * NR:3 * NR]
            c_amask = cB[:, 3 * NR:4 * NR]

            # ---- memset + derived consts (DVE, early) ----
            wsrc = sb.tile([128, WN], BF16)
            nc.vector.memset(wsrc, 0.0)
            onescol = sb.tile([128, 1], BF16)
            nc.vector.memset(onescol, 1.0)
            neghcol = sb.tile([128, 1], BF16)
            nc.vector.memset(neghcol, -0.5)
            ones300 = sb.tile([1, NQT], BF16)
            nc.vector.memset(ones300, 1.0)

            # ---- PE warmup: release the HAM clock gate before real work ----
            w_ps = wp.tile([128, WN], F32, space="PSUM")
            for i in range(N_WARM):
                nc.tensor.matmul(w_ps, lhsT=wsrc[:, 0:128], rhs=wsrc,
                                 start=True, stop=True)

            # derived const matrices (need cB)
            d316 = sb.tile([NR, NR], BF16)
            nc.vector.tensor_scalar(d316, c_I, 316.0, None, ALU.mult)
            d12 = sb.tile([NR, NR], BF16)
            nc.vector.tensor_scalar(d12, c_I, 12.0, None, ALU.mult)
            twoI = sb.tile([NR, NR], BF16)
            nc.vector.tensor_scalar(twoI, c_I, 2.0, None, ALU.mult)

            # ---- R = support + noise on DVE (bf16) ----
            HD = D // 2
            rb = sb.tile([NR, D], BF16)
            for h in range(2):
                sl = slice(h * HD, (h + 1) * HD)
                nc.vector.tensor_add(rb[:, sl], sn_sb[:, sl], nz_sb[:, sl])

            # ---- sq = qt.^2 (Scalar, 2 halves) for ||q||^2 ----
            sq = sb.tile([128, NCH * NQT], BF16)
            HQ = NCH * NQT // 2
            for h in range(2):
                sl = slice(h * HQ, (h + 1) * HQ)
                nc.scalar.activation(sq[:, sl], qtb[:, sl], AF.Square)

            # ---- RcT = (H R)^T by chunks (bf16) ----
            rctb = sb.tile([128, NCH * NR], BF16)
            for p in range(2):
                rct_ps = pipe.tile([128, 4 * NR], F32, space="PSUM", tag="pp")
                for kk in range(4):
                    k = 4 * p + kk
                    nc.tensor.matmul(rct_ps[:, kk * NR:(kk + 1) * NR],
                                     lhsT=rb[:, k * 128:(k + 1) * 128],
                                     rhs=c_Hb, start=True, stop=True)
                nc.vector.tensor_copy(rctb[:, p * 4 * NR:(p + 1) * 4 * NR],
                                      rct_ps)

            # ---- G + lam*I in one psum (diag matmuls are exact) ----
            g_ps = gp.tile([NR, NR], F32, space="PSUM")
            nc.tensor.matmul(g_ps, lhsT=d316, rhs=d316, start=True, stop=False)
            nc.tensor.matmul(g_ps, lhsT=d12, rhs=d12, start=False, stop=False)
            for k in range(NCH):
                rct_k = rctb[:, k * NR:(k + 1) * NR]
                nc.tensor.matmul(g_ps, lhsT=rct_k, rhs=rct_k,
                                 start=False, stop=(k == NCH - 1))

            # ---- qn2 = -0.5*||q||^2 row [1,300] via ones^T sq ----
            qn_ps = qnp.tile([1, NQT], F32, space="PSUM")
            for k in range(NCH):
                nc.tensor.matmul(qn_ps, lhsT=onescol,
                                 rhs=sq[:, k * NQT:(k + 1) * NQT],
                                 start=(k == 0), stop=(k == NCH - 1))
            qn_f = sb.tile([1, NQT], F32)
            nc.vector.tensor_scalar(qn_f, qn_ps, -0.5, None, ALU.mult)
            qnh_b = sb.tile([1, NQT], BF16)
            nc.scalar.activation(qnh_b, qn_ps, AF.Copy, scale=-0.5)
            qnh_f = sb.tile([1, NQT], F32)
            nc.scalar.copy(qnh_f, qnh_b)
            qnr_b = sb.tile([1, NQT], BF16)
            nc.vector.tensor_sub(qnr_b, qn_f, qnh_f)

            # ---- Ka (masked), one Newton step, W^T ----
            ka_b = sb.tile([NR, NR], BF16)
            nc.vector.tensor_mul(ka_b, g_ps, c_amask)      # Ka = amask*(G+lamI)
            y1_b = sb.tile([NR, NR], BF16)
            nc.vector.tensor_sub(y1_b, twoI, ka_b)         # Y1 = 2I - Ka
            p_ps = pipe.tile([NR, NR], F32, space="PSUM", tag="pp")
            nc.tensor.matmul(p_ps, lhsT=ka_b, rhs=y1_b, start=True, stop=True)
            qq_b = sb.tile([NR, NR], BF16)
            nc.vector.tensor_sub(qq_b, twoI, p_ps)         # 2I - Ka Y1
            y2_ps = pipe.tile([NR, NR], F32, space="PSUM", tag="pp")
            nc.tensor.matmul(y2_ps, lhsT=y1_b, rhs=qq_b, start=True, stop=True)
            y2_b = sb.tile([NR, NR], BF16)
            nc.scalar.copy(y2_b, y2_ps)
            hy_ps = pipe.tile([NR, NR], F32, space="PSUM", tag="pp")
            nc.tensor.matmul(hy_ps, lhsT=c_alHb, rhs=y2_b, start=True,
                             stop=True)
            wt_b = sb.tile([NR, NR], BF16)
            nc.vector.tensor_sub(wt_b, c_I, hy_ps)         # W^T = I - alH Y

            # ---- C^T chunks (bf16) + squares for ||C||^2 ----
            ctb = sb.tile([128, NCH * NR], BF16)
            csqb = sb.tile([128, NCH * NR], BF16)
            for p in range(2):
                ct_ps = pipe.tile([128, 4 * NR], F32, space="PSUM", tag="pp")
                for kk in range(4):
                    k = 4 * p + kk
                    nc.tensor.matmul(ct_ps[:, kk * NR:(kk + 1) * NR],
                                     lhsT=rb[:, k * 128:(k + 1) * 128],
                                     rhs=wt_b, start=True, stop=True)
                sl = slice(p * 4 * NR, (p + 1) * 4 * NR)
                nc.vector.tensor_copy(ctb[:, sl], ct_ps)
                nc.scalar.activation(csqb[:, sl], ct_ps, AF.Square)

            # ---- cn row: [1,100] = sum_d -0.5 * C^T(d,j)^2 (fp32) ----
            cn_ps = cnp.tile([1, NR], F32, space="PSUM")
            for k in range(NCH):
                nc.tensor.matmul(cn_ps, lhsT=neghcol,
                                 rhs=csqb[:, k * NR:(k + 1) * NR],
                                 start=(k == 0), stop=(k == NCH - 1))
            cn_f = sb.tile([1, NR], F32)
            nc.vector.tensor_copy(cn_f, cn_ps)             # DVE ...
            cnh_b = sb.tile([1, NR], BF16)
            nc.scalar.copy(cnh_b, cn_ps)                   # ... || ACT
            cnh_f = sb.tile([1, NR], F32)
            nc.scalar.copy(cnh_f, cnh_b)
            cnr_b = sb.tile([1, NR], BF16)
            nc.vector.tensor_sub(cnr_b, cn_f, cnh_f)

            # ---- QC transposed: psum[(t,j),(t,q)] = C q^T + norm folds ----
            ones100 = ones300[0:1, 0:NR]
            qc_ps = qcp.tile([NR, NQT], F32, space="PSUM")
            nc.tensor.matmul(qc_ps, lhsT=ones100, rhs=qnh_b,
                             start=True, stop=False)
            nc.tensor.matmul(qc_ps, lhsT=ones100, rhs=qnr_b,
                             start=False, stop=False)
            for k in range(NCH):
                nc.tensor.matmul(qc_ps, lhsT=ctb[:, k * NR:(k + 1) * NR],
                                 rhs=qtb[:, k * NQT:(k + 1) * NQT],
                                 start=False, stop=False)
            nc.tensor.matmul(qc_ps, lhsT=cnh_b, rhs=ones300,
                             start=False, stop=False)
            nc.tensor.matmul(qc_ps, lhsT=cnr_b, rhs=ones300,
                             start=False, stop=True)

            # ---- epilogue: scale full psum, DMA out; host slices blocks ----
            out_sb = sb.tile([NR, NQT], F32)
            nc.vector.tensor_scalar(out_sb, qc_ps, 2.0 / D, None, ALU.mult)
            nc.sync.dma_start(out=out_d[:], in_=out_sb)

    nc.finalize()
    return nc


_NC_CACHE = None


def _get_nc():
    global _NC_CACHE
    if _NC_CACHE is None:
        _NC_CACHE = build_nc()
    return _NC_CACHE


def make_in_maps(query, support, noise):
    query = np.asarray(query, dtype=np.float32)
    support = np.asarray(support, dtype=np.float32)
    noise = np.asarray(noise, dtype=np.float32)
    cB = _host_consts()
    in_maps = []
    for c in range(N_CORES):
        ts = slice(c * TPC, (c + 1) * TPC)
        qc = query[ts]                                   # (4, 75, 1024)
        # qt[p, k*300 + t*75 + q] = q[t, q, 128k+p]
        qt = np.ascontiguousarray(
            qc.transpose(2, 0, 1).reshape(NCH, 128, NQT)
              .transpose(1, 0, 2).reshape(128, NCH * NQT)
        ).astype(ml_dtypes.float8_e4m3)
        in_maps.append({
            "qt": qt,
            "sn": np.ascontiguousarray(support[ts]).reshape(NR, D)
                  .astype(ml_dtypes.bfloat16),
            "nz": np.ascontiguousarray(
                noise[:, ts].transpose(1, 0, 2, 3)).reshape(NR, D)
                  .astype(ml_dtypes.bfloat16),
            "cB": cB,
        })
    return in_maps


def kernel(query, support, noise, support_labels=None, n_way=None, n_shot=None,
           **_unused):
    nc = _get_nc()
    in_maps = make_in_maps(query, support, noise)
    res = run_bass_kernel_spmd(nc, in_maps, list(range(N_CORES)))
    # out is [(t,j), (t',q)] = [100, 300]; take diagonal task blocks,
    # then (4, 25, 75) -> (4, 75, 25)
    outs = []
    for r in res.results:
        o = np.asarray(r["out"]).reshape(TPC, NJ, TPC, NQ)
        blk = o[np.arange(TPC), :, np.arange(TPC), :]   # (4, 25, 75)
        outs.append(blk.transpose(0, 2, 1))
    full = np.concatenate(outs, axis=0)            # (32, 75, 25)
    return full.reshape(T_FULL, NQ, NW, NS).astype(np.float32)


# revision 7
# speedup vs baseline: 1.4286x; 1.0167x over previous
"""Trainium2 Bass kernel for the ExemplarHead classification problem (v4).

Math: per (task, way), with R the 5x1024 class reps (support+noise),
H = I - (1/5)11^T, G = H R R^T H, the SVD head reduces exactly to
    C = W R,  W = I - lam * (lam I + G)^{-1} H
    logits[q,(w,s)] = (2 q.C - ||q||^2 - ||C||^2) / d
(lam I + G) inverse via one scaled Newton step. All 20 (task,way) blocks
per core are one masked block-diagonal 100x100 problem.

v5 changes vs v4 (31.4us measured):
 - DMA rebalanced across all three rings: sn+cB on sync, nz on scalar
   (was idle), qt-cast on gpsimd -> inputs land ~10.5us not 12.8us.
 - warmup tuned so the PE HAM clock-gate stays released into the real
   matmul stream (v4 had a 3.3us PE idle gap -> tail ran at 1.2GHz).
 - ||q||^2 / ||C||^2 folds use single fp16 rank-1 matmuls (11-bit
   mantissa) instead of bf16+residual pairs: 6 fewer serial tail ops.

Sharding: data-parallel over the 32 tasks -> 4 tasks per NeuronCore x 8.
"""

import numpy as np
import ml_dtypes

import concourse.bass as bass
import concourse.mybir as mybir
import concourse.tile as tile
from concourse import bacc
from concourse.bass_utils import run_bass_kernel_spmd

F32 = mybir.dt.float32
BF16 = mybir.dt.bfloat16
FP16 = mybir.dt.float16
FP8 = mybir.dt.float8e4
AF = mybir.ActivationFunctionType
ALU = mybir.AluOpType

LAM = 100000.0
GMAX_BOUND = 40000.0            # safe bound on ||G|| (observed max ~2.2e4)
ALPHA = 2.0 / (2.0 * LAM + GMAX_BOUND)

N_CORES = 8
T_FULL, NQ, D = 32, 75, 1024
NW, NS = 5, 5
TPC = T_FULL // N_CORES          # tasks per core = 4
NR = TPC * NW * NS               # R rows per core = 100
NCH = D // 128                   # 8 contraction chunks
NJ = NW * NS                     # 25 (way,shot) pairs per task
NQT = TPC * NQ                   # 300 (task,query) columns per core
CB_COLS = 400                    # bf16 const tile columns
N_WARM = 7                      # PE warmup matmuls
WN = 384                         # warmup matmul free size


def _host_consts():
    """cB bf16 [100,400]: H (block-diag), alpha*lam*H, I, alpha*blockmask."""
    H5 = np.eye(NS) - np.ones((NS, NS)) / NS
    H_bd = np.kron(np.eye(TPC * NW), H5).astype(np.float32)       # [100,100]
    blockmask = np.kron(np.eye(TPC * NW), np.ones((NS, NS))).astype(np.float32)
    eye = np.eye(NR, dtype=np.float32)
    cB = np.zeros((NR, CB_COLS), dtype=np.float32)
    cB[:, 0:NR] = H_bd
    cB[:, NR:2 * NR] = ALPHA * LAM * H_bd
    cB[:, 2 * NR:3 * NR] = eye
    cB[:, 3 * NR:4 * NR] = ALPHA * blockmask
    return cB.astype(ml_dtypes.bfloat16)


def build_nc():
    nc = bacc.Bacc("TRN2")

    qt_d = nc.declare_dram_parameter("qt", [128, NCH * NQT], FP8,
                                     isOutput=False)
    sn_d = nc.declare_dram_parameter("sn", [NR, D], BF16, isOutput=False)
    nz_d = nc.declare_dram_parameter("nz", [NR, D], BF16, isOutput=False)
    cB_d = nc.declare_dram_parameter("cB", [NR, CB_COLS], BF16, isOutput=False)
    out_d = nc.declare_dram_parameter("out", [NR, NQT], F32, isOutput=True)

    with tile.TileContext(nc) as tc:
        with (
            tc.tile_pool(name="consts", bufs=1) as consts,
            tc.tile_pool(name="sb", bufs=1) as sb,
            tc.tile_pool(name="pipe", bufs=3, space="PSUM") as pipe,
            tc.tile_pool(name="gp", bufs=1, space="PSUM") as gp,
            tc.tile_pool(name="cnp", bufs=1, space="PSUM") as cnp,
            tc.tile_pool(name="qnp", bufs=1, space="PSUM") as qnp,
            tc.tile_pool(name="qcp", bufs=1, space="PSUM") as qcp,
            tc.tile_pool(name="wp", bufs=1, space="PSUM") as wp,
        ):
            # ---- input DMAs: 3 parallel paths ----
            cB = consts.tile([NR, CB_COLS], BF16)
            sn_sb = sb.tile([NR, D], BF16)
            nz_sb = sb.tile([NR, D], BF16)
            nc.sync.dma_start(out=sn_sb, in_=sn_d[:])
            nc.scalar.dma_start(out=nz_sb, in_=nz_d[:])
            nc.sync.dma_start(out=cB, in_=cB_d[:])
            qtb = sb.tile([128, NCH * NQT], BF16)
            nc.gpsimd.dma_start(out=qtb, in_=qt_d[:])      # SWDGE fp8->bf16
            c_Hb = cB[:, 0:NR]
            c_alHb = cB[:, NR:2 * NR]
            c_I = cB[:, 2 * NR:3 * NR]
            c_amask = cB[:, 3 * NR:4 * NR]

            # ---- memset + derived consts (DVE, early) ----
            wsrc = sb.tile([128, WN], BF16)
            nc.vector.memset(wsrc, 0.0)
            onescol = sb.tile([128, 1], BF16)
            nc.vector.memset(onescol, 1.0)
            neghcol = sb.tile([128, 1], BF16)
            nc.vector.memset(neghcol, -0.5)
            ones300 = sb.tile([1, NQT], FP16)
            nc.vector.memset(ones300, 1.0)

            # ---- PE warmup: release the HAM clock gate before real work ----
            w_ps = wp.tile([128, WN], F32, space="PSUM")
            for i in range(N_WARM):
                nc.tensor.matmul(w_ps, lhsT=wsrc[:, 0:128], rhs=wsrc,
                                 start=True, stop=True)

            # ---- R = support + noise on DVE (bf16, quarters) ----
            QD = D // 4
            rb = sb.tile([NR, D], BF16)
            for h in range(4):
                sl = slice(h * QD, (h + 1) * QD)
                nc.vector.tensor_add(rb[:, sl], sn_sb[:, sl], nz_sb[:, sl])

            # derived const matrices (need cB)
            d316 = sb.tile([NR, NR], BF16)
            nc.vector.tensor_scalar(d316, c_I, 316.0, None, ALU.mult)
            d12 = sb.tile([NR, NR], BF16)
            nc.vector.tensor_scalar(d12, c_I, 12.0, None, ALU.mult)
            twoI = sb.tile([NR, NR], BF16)
            nc.vector.tensor_scalar(twoI, c_I, 2.0, None, ALU.mult)

            # ---- sq = qt.^2 (Scalar, 2 halves) for ||q||^2 ----
            sq = sb.tile([128, NCH * NQT], BF16)
            HQ = NCH * NQT // 2
            for h in range(2):
                sl = slice(h * HQ, (h + 1) * HQ)
                nc.scalar.activation(sq[:, sl], qtb[:, sl], AF.Square)

            # ---- RcT = (H R)^T by chunks (bf16) ----
            rctb = sb.tile([128, NCH * NR], BF16)
            for p in range(2):
                rct_ps = pipe.tile([128, 4 * NR], F32, space="PSUM", tag="pp")
                for kk in range(4):
                    k = 4 * p + kk
                    nc.tensor.matmul(rct_ps[:, kk * NR:(kk + 1) * NR],
                                     lhsT=rb[:, k * 128:(k + 1) * 128],
                                     rhs=c_Hb, start=True, stop=True)
                nc.vector.tensor_copy(rctb[:, p * 4 * NR:(p + 1) * 4 * NR],
                                      rct_ps)

            # ---- G + lam*I in one psum (diag matmuls are exact) ----
            g_ps = gp.tile([NR, NR], F32, space="PSUM")
            nc.tensor.matmul(g_ps, lhsT=d316, rhs=d316, start=True, stop=False)
            nc.tensor.matmul(g_ps, lhsT=d12, rhs=d12, start=False, stop=False)
            for k in range(NCH):
                rct_k = rctb[:, k * NR:(k + 1) * NR]
                nc.tensor.matmul(g_ps, lhsT=rct_k, rhs=rct_k,
                                 start=False, stop=(k == NCH - 1))

            # ---- qn2 = -0.5*||q||^2 row [1,300] via ones^T sq ----
            qn_ps = qnp.tile([1, NQT], F32, space="PSUM")
            for k in range(NCH):
                nc.tensor.matmul(qn_ps, lhsT=onescol,
                                 rhs=sq[:, k * NQT:(k + 1) * NQT],
                                 start=(k == 0), stop=(k == NCH - 1))
            qnh = sb.tile([1, NQT], FP16)
            nc.scalar.activation(qnh, qn_ps, AF.Copy, scale=-0.5)

            # ---- Ka (masked), one Newton step, W^T ----
            ka_b = sb.tile([NR, NR], BF16)
            nc.vector.tensor_mul(ka_b, g_ps, c_amask)      # Ka = amask*(G+lamI)
            y1_b = sb.tile([NR, NR], BF16)
            nc.vector.tensor_sub(y1_b, twoI, ka_b)         # Y1 = 2I - Ka
            p_ps = pipe.tile([NR, NR], F32, space="PSUM", tag="pp")
            nc.tensor.matmul(p_ps, lhsT=ka_b, rhs=y1_b, start=True, stop=True)
            qq_b = sb.tile([NR, NR], BF16)
            nc.vector.tensor_sub(qq_b, twoI, p_ps)         # 2I - Ka Y1
            y2_ps = pipe.tile([NR, NR], F32, space="PSUM", tag="pp")
            nc.tensor.matmul(y2_ps, lhsT=y1_b, rhs=qq_b, start=True, stop=True)
            y2_b = sb.tile([NR, NR], BF16)
            nc.scalar.copy(y2_b, y2_ps)
            hy_ps = pipe.tile([NR, NR], F32, space="PSUM", tag="pp")
            nc.tensor.matmul(hy_ps, lhsT=c_alHb, rhs=y2_b, start=True,
                             stop=True)
            wt_b = sb.tile([NR, NR], BF16)
            nc.vector.tensor_sub(wt_b, c_I, hy_ps)         # W^T = I - alH Y

            # ---- C^T chunks (bf16) + squares for ||C||^2 ----
            ctb = sb.tile([128, NCH * NR], BF16)
            csqb = sb.tile([128, NCH * NR], BF16)
            for p in range(2):
                ct_ps = pipe.tile([128, 4 * NR], F32, space="PSUM", tag="pp")
                for kk in range(4):
                    k = 4 * p + kk
                    nc.tensor.matmul(ct_ps[:, kk * NR:(kk + 1) * NR],
                                     lhsT=rb[:, k * 128:(k + 1) * 128],
                                     rhs=wt_b, start=True, stop=True)
                sl = slice(p * 4 * NR, (p + 1) * 4 * NR)
                nc.vector.tensor_copy(ctb[:, sl], ct_ps)
                nc.scalar.activation(csqb[:, sl], ct_ps, AF.Square)

            # ---- cn row: [1,100] = sum_d -0.5 * C^T(d,j)^2 (fp32) ----
            cn_ps = cnp.tile([1, NR], F32, space="PSUM")
            for k in range(NCH):
                nc.tensor.matmul(cn_ps, lhsT=neghcol,
                                 rhs=csqb[:, k * NR:(k + 1) * NR],
                                 start=(k == 0), stop=(k == NCH - 1))
            cnh = sb.tile([1, NR], FP16)
            nc.scalar.copy(cnh, cn_ps)

            # ---- QC transposed: psum[(t,j),(t,q)] = C q^T + norm folds ----
            ones100 = ones300[0:1, 0:NR]
            qc_ps = qcp.tile([NR, NQT], F32, space="PSUM")
            nc.tensor.matmul(qc_ps, lhsT=ones100, rhs=qnh,
                             start=True, stop=False)
            for k in range(NCH):
                nc.tensor.matmul(qc_ps, lhsT=ctb[:, k * NR:(k + 1) * NR],
                                 rhs=qtb[:, k * NQT:(k + 1) * NQT],
                                 start=False, stop=False)
            nc.tensor.matmul(qc_ps, lhsT=cnh, rhs=ones300,
                             start=False, stop=True)

            # ---- epilogue: scale full psum, DMA out; host slices blocks ----
            out_sb = sb.tile([NR, NQT], F32)
            nc.vector.tensor_scalar(out_sb, qc_ps, 2.0 / D, None, ALU.mult)
            nc.sync.dma_start(out=out_d[:], in_=out_sb)

    nc.finalize()
    return nc


_NC_CACHE = None


def _get_nc():
    global _NC_CACHE
    if _NC_CACHE is None:
        _NC_CACHE = build_nc()
    return _NC_CACHE


def make_in_maps(query, support, noise):
    query = np.asarray(query, dtype=np.float32)
    support = np.asarray(support, dtype=np.float32)
    noise = np.asarray(noise, dtype=np.float32)
    cB = _host_consts()
    in_maps = []
    for c in range(N_CORES):
        ts = slice(c * TPC, (c + 1) * TPC)
        qc = query[ts]                                   # (4, 75, 1024)
        # qt[p, k*300 + t*75 + q] = q[t, q, 128k+p]
        qt = np.ascontiguousarray(
            qc.transpose(2, 0, 1).reshape(NCH, 128, NQT)
              .transpose(1, 0, 2).reshape(128, NCH * NQT)
        ).astype(ml_dtypes.float8_e4m3)
        in_maps.append({
            "qt": qt,
            "sn": np.ascontiguousarray(support[ts]).reshape(NR, D)
                  .astype(ml_dtypes.bfloat16),
            "nz": np.ascontiguousarray(
                noise[:, ts].transpose(1, 0, 2, 3)).reshape(NR, D)
                  .astype(ml_dtypes.bfloat16),
            "cB": cB,
        })
    return in_maps


def kernel(query, support, noise, support_labels=None, n_way=None, n_shot=None,
           **_unused):
    nc = _get_nc()
    in_maps = make_in_maps(query, support, noise)
    res = run_bass_kernel_spmd(nc, in_maps, list(range(N_CORES)))
    # out is [(t,j), (t',q)] = [100, 300]; take diagonal task blocks,
    # then (4, 25, 75) -> (4, 75, 25)
    outs = []
    for r in res.results:
        o = np.asarray(r["out"]).reshape(TPC, NJ, TPC, NQ)
        blk = o[np.arange(TPC), :, np.arange(TPC), :]   # (4, 25, 75)
        outs.append(blk.transpose(0, 2, 1))
    full = np.concatenate(outs, axis=0)            # (32, 75, 25)
    return full.reshape(T_FULL, NQ, NW, NS).astype(np.float32)


# revision 8
# speedup vs baseline: 1.4628x; 1.0240x over previous
"""Trainium2 Bass kernel for the ExemplarHead classification problem (v4).

Math: per (task, way), with R the 5x1024 class reps (support+noise),
H = I - (1/5)11^T, G = H R R^T H, the SVD head reduces exactly to
    C = W R,  W = I - lam * (lam I + G)^{-1} H
    logits[q,(w,s)] = (2 q.C - ||q||^2 - ||C||^2) / d
(lam I + G) inverse via one scaled Newton step. All 20 (task,way) blocks
per core are one masked block-diagonal 100x100 problem.

v5 changes vs v4 (31.4us measured):
 - DMA rebalanced across all three rings: sn+cB on sync, nz on scalar
   (was idle), qt-cast on gpsimd -> inputs land ~10.5us not 12.8us.
 - warmup tuned so the PE HAM clock-gate stays released into the real
   matmul stream (v4 had a 3.3us PE idle gap -> tail ran at 1.2GHz).
 - ||q||^2 / ||C||^2 folds use single fp16 rank-1 matmuls (11-bit
   mantissa) instead of bf16+residual pairs: 6 fewer serial tail ops.

Sharding: data-parallel over the 32 tasks -> 4 tasks per NeuronCore x 8.
"""

import numpy as np
import ml_dtypes

import concourse.bass as bass
import concourse.mybir as mybir
import concourse.tile as tile
from concourse import bacc
from concourse.bass_utils import run_bass_kernel_spmd

F32 = mybir.dt.float32
BF16 = mybir.dt.bfloat16
FP16 = mybir.dt.float16
FP8 = mybir.dt.float8e4
AF = mybir.ActivationFunctionType
ALU = mybir.AluOpType

LAM = 100000.0
GMAX_BOUND = 40000.0            # safe bound on ||G|| (observed max ~2.2e4)
ALPHA = 2.0 / (2.0 * LAM + GMAX_BOUND)

N_CORES = 8
T_FULL, NQ, D = 32, 75, 1024
NW, NS = 5, 5
TPC = T_FULL // N_CORES          # tasks per core = 4
NR = TPC * NW * NS               # R rows per core = 100
NCH = D // 128                   # 8 contraction chunks
NJ = NW * NS                     # 25 (way,shot) pairs per task
NQT = TPC * NQ                   # 300 (task,query) columns per core
CB_COLS = 400                    # bf16 const tile columns
N_WARM = 12                      # PE warmup matmuls
WN = 384                         # warmup matmul free size


def _host_consts():
    """cB bf16 [100,400]: H (block-diag), alpha*lam*H, I, alpha*blockmask."""
    H5 = np.eye(NS) - np.ones((NS, NS)) / NS
    H_bd = np.kron(np.eye(TPC * NW), H5).astype(np.float32)       # [100,100]
    blockmask = np.kron(np.eye(TPC * NW), np.ones((NS, NS))).astype(np.float32)
    eye = np.eye(NR, dtype=np.float32)
    cB = np.zeros((NR, CB_COLS), dtype=np.float32)
    cB[:, 0:NR] = H_bd
    cB[:, NR:2 * NR] = ALPHA * LAM * H_bd
    cB[:, 2 * NR:3 * NR] = eye
    cB[:, 3 * NR:4 * NR] = ALPHA * blockmask
    return cB.astype(ml_dtypes.bfloat16)


def build_nc():
    nc = bacc.Bacc("TRN2")

    qt_d = nc.declare_dram_parameter("qt", [128, NCH * NQT], FP8,
                                     isOutput=False)
    sn_d = nc.declare_dram_parameter("sn", [NR, D], BF16, isOutput=False)
    nz_d = nc.declare_dram_parameter("nz", [NR, D], BF16, isOutput=False)
    cB_d = nc.declare_dram_parameter("cB", [NR, CB_COLS], BF16, isOutput=False)
    out_d = nc.declare_dram_parameter("out", [NR, NQT], F32, isOutput=True)

    with tile.TileContext(nc) as tc:
        with (
            tc.tile_pool(name="consts", bufs=1) as consts,
            tc.tile_pool(name="sb", bufs=1) as sb,
            tc.tile_pool(name="pipe", bufs=3, space="PSUM") as pipe,
            tc.tile_pool(name="gp", bufs=1, space="PSUM") as gp,
            tc.tile_pool(name="cnp", bufs=1, space="PSUM") as cnp,
            tc.tile_pool(name="qnp", bufs=1, space="PSUM") as qnp,
            tc.tile_pool(name="qcp", bufs=1, space="PSUM") as qcp,
            tc.tile_pool(name="wp", bufs=1, space="PSUM") as wp,
        ):
            # ---- input DMAs: 3 parallel paths ----
            cB = consts.tile([NR, CB_COLS], BF16)
            sn_sb = sb.tile([NR, D], BF16)
            nz_sb = sb.tile([NR, D], BF16)
            nc.sync.dma_start(out=sn_sb, in_=sn_d[:])
            nc.scalar.dma_start(out=nz_sb, in_=nz_d[:])
            nc.sync.dma_start(out=cB, in_=cB_d[:])
            qtb = sb.tile([128, NCH * NQT], BF16)
            nc.gpsimd.dma_start(out=qtb, in_=qt_d[:])      # SWDGE fp8->bf16
            c_Hb = cB[:, 0:NR]
            c_alHb = cB[:, NR:2 * NR]
            c_I = cB[:, 2 * NR:3 * NR]
            c_amask = cB[:, 3 * NR:4 * NR]

            # ---- memset + derived consts (DVE, early) ----
            wsrc = sb.tile([128, WN], BF16)
            nc.vector.memset(wsrc, 0.0)
            onescol = sb.tile([128, 1], BF16)
            nc.vector.memset(onescol, 1.0)
            neghcol = sb.tile([128, 1], BF16)
            nc.vector.memset(neghcol, -0.5)
            ones300 = sb.tile([1, NQT], FP16)
            nc.vector.memset(ones300, 1.0)

            # ---- PE warmup: release the HAM clock gate before real work ----
            w_ps = wp.tile([128, WN], F32, space="PSUM")
            for i in range(N_WARM):
                nc.tensor.matmul(w_ps, lhsT=wsrc[:, 0:128], rhs=wsrc,
                                 start=True, stop=True)

            # ---- R = support + noise on DVE (bf16, quarters) ----
            QD = D // 4
            rb = sb.tile([NR, D], BF16)
            for h in range(4):
                sl = slice(h * QD, (h + 1) * QD)
                nc.vector.tensor_add(rb[:, sl], sn_sb[:, sl], nz_sb[:, sl])

            # derived const matrices (need cB)
            d316 = sb.tile([NR, NR], BF16)
            nc.vector.tensor_scalar(d316, c_I, 316.0, None, ALU.mult)
            d12 = sb.tile([NR, NR], BF16)
            nc.vector.tensor_scalar(d12, c_I, 12.0, None, ALU.mult)
            twoI = sb.tile([NR, NR], BF16)
            nc.vector.tensor_scalar(twoI, c_I, 2.0, None, ALU.mult)

            # ---- sq = qt.^2 (Scalar, 2 halves) for ||q||^2 ----
            sq = sb.tile([128, NCH * NQT], BF16)
            HQ = NCH * NQT // 2
            for h in range(2):
                sl = slice(h * HQ, (h + 1) * HQ)
                nc.scalar.activation(sq[:, sl], qtb[:, sl], AF.Square)

            # ---- RcT = (H R)^T by chunks (bf16) ----
            rctb = sb.tile([128, NCH * NR], BF16)
            for p in range(2):
                rct_ps = pipe.tile([128, 4 * NR], F32, space="PSUM", tag="pp")
                for kk in range(4):
                    k = 4 * p + kk
                    nc.tensor.matmul(rct_ps[:, kk * NR:(kk + 1) * NR],
                                     lhsT=rb[:, k * 128:(k + 1) * 128],
                                     rhs=c_Hb, start=True, stop=True)
                nc.vector.tensor_copy(rctb[:, p * 4 * NR:(p + 1) * 4 * NR],
                                      rct_ps)

            # ---- G + lam*I in one psum (diag matmuls are exact) ----
            g_ps = gp.tile([NR, NR], F32, space="PSUM")
            nc.tensor.matmul(g_ps, lhsT=d316, rhs=d316, start=True, stop=False)
            nc.tensor.matmul(g_ps, lhsT=d12, rhs=d12, start=False, stop=False)
            for k in range(NCH):
                rct_k = rctb[:, k * NR:(k + 1) * NR]
                nc.tensor.matmul(g_ps, lhsT=rct_k, rhs=rct_k,
                                 start=False, stop=(k == NCH - 1))

            # ---- Ka (masked), one Newton step, W^T ----
            ka_b = sb.tile([NR, NR], BF16)
            nc.vector.tensor_mul(ka_b, g_ps, c_amask)      # Ka = amask*(G+lamI)
            y1_b = sb.tile([NR, NR], BF16)
            nc.vector.tensor_sub(y1_b, twoI, ka_b)         # Y1 = 2I - Ka
            p_ps = pipe.tile([NR, NR], F32, space="PSUM", tag="pp")
            nc.tensor.matmul(p_ps, lhsT=ka_b, rhs=y1_b, start=True, stop=True)
            qq_b = sb.tile([NR, NR], BF16)
            nc.vector.tensor_sub(qq_b, twoI, p_ps)         # 2I - Ka Y1
            y2_ps = pipe.tile([NR, NR], F32, space="PSUM", tag="pp")
            nc.tensor.matmul(y2_ps, lhsT=y1_b, rhs=qq_b, start=True, stop=True)
            y2_b = sb.tile([NR, NR], BF16)
            nc.scalar.copy(y2_b, y2_ps)
            hy_ps = pipe.tile([NR, NR], F32, space="PSUM", tag="pp")
            nc.tensor.matmul(hy_ps, lhsT=c_alHb, rhs=y2_b, start=True,
                             stop=True)
            wt_b = sb.tile([NR, NR], BF16)
            nc.vector.tensor_sub(wt_b, c_I, hy_ps)         # W^T = I - alH Y

            # ---- qn2 = -0.5*||q||^2 row [1,300] via ones^T sq ----
            qn_ps = qnp.tile([1, NQT], F32, space="PSUM")
            for k in range(NCH):
                nc.tensor.matmul(qn_ps, lhsT=onescol,
                                 rhs=sq[:, k * NQT:(k + 1) * NQT],
                                 start=(k == 0), stop=(k == NCH - 1))
            qnh = sb.tile([1, NQT], FP16)
            nc.scalar.activation(qnh, qn_ps, AF.Copy, scale=-0.5)


            # ---- C^T chunks (bf16) + squares for ||C||^2 ----
            ctb = sb.tile([128, NCH * NR], BF16)
            csqb = sb.tile([128, NCH * NR], BF16)
            for p in range(2):
                ct_ps = pipe.tile([128, 4 * NR], F32, space="PSUM", tag="pp")
                for kk in range(4):
                    k = 4 * p + kk
                    nc.tensor.matmul(ct_ps[:, kk * NR:(kk + 1) * NR],
                                     lhsT=rb[:, k * 128:(k + 1) * 128],
                                     rhs=wt_b, start=True, stop=True)
                sl = slice(p * 4 * NR, (p + 1) * 4 * NR)
                nc.vector.tensor_copy(ctb[:, sl], ct_ps)
                nc.scalar.activation(csqb[:, sl], ct_ps, AF.Square)

            # ---- cn row: [1,100] = sum_d -0.5 * C^T(d,j)^2 (fp32) ----
            cn_ps = cnp.tile([1, NR], F32, space="PSUM")
            for k in range(NCH):
                nc.tensor.matmul(cn_ps, lhsT=neghcol,
                                 rhs=csqb[:, k * NR:(k + 1) * NR],
                                 start=(k == 0), stop=(k == NCH - 1))
            cnh = sb.tile([1, NR], FP16)
            nc.scalar.copy(cnh, cn_ps)

            # ---- QC transposed: psum[(t,j),(t,q)] = C q^T + norm folds ----
            ones100 = ones300[0:1, 0:NR]
            qc_ps = qcp.tile([NR, NQT], F32, space="PSUM")
            nc.tensor.matmul(qc_ps, lhsT=ones100, rhs=qnh,
                             start=True, stop=False)
            for k in range(NCH):
                nc.tensor.matmul(qc_ps, lhsT=ctb[:, k * NR:(k + 1) * NR],
                                 rhs=qtb[:, k * NQT:(k + 1) * NQT],
                                 start=False, stop=False)
            nc.tensor.matmul(qc_ps, lhsT=cnh, rhs=ones300,
                             start=False, stop=True)

            # ---- epilogue: scale full psum, DMA out; host slices blocks ----
            out_sb = sb.tile([NR, NQT], F32)
            nc.vector.tensor_scalar(out_sb, qc_ps, 2.0 / D, None, ALU.mult)
            nc.sync.dma_start(out=out_d[:], in_=out_sb)

    nc.finalize()
    return nc


_NC_CACHE = None


def _get_nc():
    global _NC_CACHE
    if _NC_CACHE is None:
        _NC_CACHE = build_nc()
    return _NC_CACHE


def make_in_maps(query, support, noise):
    query = np.asarray(query, dtype=np.float32)
    support = np.asarray(support, dtype=np.float32)
    noise = np.asarray(noise, dtype=np.float32)
    cB = _host_consts()
    in_maps = []
    for c in range(N_CORES):
        ts = slice(c * TPC, (c + 1) * TPC)
        qc = query[ts]                                   # (4, 75, 1024)
        # qt[p, k*300 + t*75 + q] = q[t, q, 128k+p]
        qt = np.ascontiguousarray(
            qc.transpose(2, 0, 1).reshape(NCH, 128, NQT)
              .transpose(1, 0, 2).reshape(128, NCH * NQT)
        ).astype(ml_dtypes.float8_e4m3)
        in_maps.append({
            "qt": qt,
            "sn": np.ascontiguousarray(support[ts]).reshape(NR, D)
                  .astype(ml_dtypes.bfloat16),
            "nz": np.ascontiguousarray(
                noise[:, ts].transpose(1, 0, 2, 3)).reshape(NR, D)
                  .astype(ml_dtypes.bfloat16),
            "cB": cB,
        })
    return in_maps


def kernel(query, support, noise, support_labels=None, n_way=None, n_shot=None,
           **_unused):
    nc = _get_nc()
    in_maps = make_in_maps(query, support, noise)
    res = run_bass_kernel_spmd(nc, in_maps, list(range(N_CORES)))
    # out is [(t,j), (t',q)] = [100, 300]; take diagonal task blocks,
    # then (4, 25, 75) -> (4, 75, 25)
    outs = []
    for r in res.results:
        o = np.asarray(r["out"]).reshape(TPC, NJ, TPC, NQ)
        blk = o[np.arange(TPC), :, np.arange(TPC), :]   # (4, 25, 75)
        outs.append(blk.transpose(0, 2, 1))
    full = np.concatenate(outs, axis=0)            # (32, 75, 25)
    return full.reshape(T_FULL, NQ, NW, NS).astype(np.float32)
